# revision 14
# baseline (speedup 1.0000x reference)
"""MultiHeadAttention Trainium2 kernel.

Full inputs -> full output. Sharding: 8 cores = (batch b in 0..3) x (head
group g in 0..1, 8 heads each). Each core projects Q/K/V for its head group
over all 2048 positions of batch b, runs attention for its 8 heads, applies
its half of the output projection (wo rows for its heads), and returns a
partial [2048, 1024] output. Host: full[b] = part(b,0) + part(b,1) + bias.

Uniform bf16 datapath (fp32 PSUM accumulation), everything SBUF-resident
between phases (no DRAM round trips) to stay under the chip power envelope:
  phase A: project Q^T, K^T [512, 2048] (rank-1 bias matmuls) and the
           V table va [keys, head, 65] with a ones column (so the softmax
           denominator falls out of the ctx matmul) scaled by
           exp(-1e9*mask) per key (exact mask semantics at zero cost).
  phase B: 32 software-pipelined (head, query-block) iterations:
           logits^T [sk, 512q] = K_h^T.T @ Q_h^T, P = exp(0.125*l) on the
           scalar engine -> bf16, ctx matmuls accumulate [65, 512] (row 64
           = denominator), DVE fast-reciprocal, PE rank-1 broadcast of the
           recips, DVE normalize-mul -> ctxn bf16 (odd heads go through a
           shifted-identity matmul to land at partitions 64..127).
  phase C: out_partial = ctxn @ wo, f32 partials to DRAM.
"""

import numpy as np
import ml_dtypes

import concourse.bass as bass
import concourse.mybir as mybir
import concourse.tile as tile
from concourse import bacc
from concourse.bass_utils import run_bass_kernel_spmd

f32 = mybir.dt.float32
bf16 = mybir.dt.bfloat16
np_bf16 = ml_dtypes.bfloat16

B, S, D, H, DH = 4, 2048, 1024, 16, 64
HG = H // 2          # 8 heads per core
DG = HG * DH         # 512 projection cols per core
N_CORES = 8
Exp = mybir.ActivationFunctionType.Exp

KC = D // 128        # 8 contraction chunks over d_model
CC = DG // 128       # 4 chunks over the head-group dim
SKC = S // 128       # 16 key chunks
NT = HG * 4          # 32 pipelined iterations (head, 512-query block)


def _build():
    nc = bacc.Bacc(None, target_bir_lowering=False)

    xq = nc.dram_tensor("xq", [D, S], bf16, kind="ExternalInput")   # query^T
    xk = nc.dram_tensor("xk", [D, S], bf16, kind="ExternalInput")   # key^T
    xv = nc.dram_tensor("xv", [D, S], bf16, kind="ExternalInput")   # value^T
    wq = nc.dram_tensor("wq", [D, DG], bf16, kind="ExternalInput")
    wk = nc.dram_tensor("wk", [D, DG], bf16, kind="ExternalInput")
    wv = nc.dram_tensor("wv", [D, DG], bf16, kind="ExternalInput")
    wo = nc.dram_tensor("wo", [DG, D], bf16, kind="ExternalInput")
    b2 = nc.dram_tensor("b2", [33, DG], bf16, kind="ExternalInput")  # bq@0, bk@32
    one = nc.dram_tensor("one", [33, 512], bf16, kind="ExternalInput")
    emask8 = nc.dram_tensor("emask8", [128, SKC, HG], bf16, kind="ExternalInput")
    emaskf = nc.dram_tensor("emaskf", [128, SKC], f32, kind="ExternalInput")
    ident = nc.dram_tensor("ident", [64, 128], bf16, kind="ExternalInput")
    out = nc.dram_tensor("out", [S, D], f32, kind="ExternalOutput")

    with tile.TileContext(nc) as tc:
        _emit(nc, tc, xq, xk, xv, wq, wk, wv, wo, b2, one, emask8, emaskf,
              ident, out)
    nc.finalize()
    return nc


def _emit(nc, tc, xq, xk, xv, wq, wk, wv, wo, b2, one, emask8, emaskf,
          ident, out):
    from contextlib import ExitStack

    with ExitStack() as ctx:
        consts = ctx.enter_context(tc.tile_pool(name="consts", bufs=1))
        wpool = ctx.enter_context(tc.tile_pool(name="wpool", bufs=2))
        xtp = ctx.enter_context(tc.tile_pool(name="xtp", bufs=2))
        big = ctx.enter_context(tc.tile_pool(name="big", bufs=1))
        ptp = ctx.enter_context(tc.tile_pool(name="ptp", bufs=16))
        tmp = ctx.enter_context(tc.tile_pool(name="tmp", bufs=1))
        rcp = ctx.enter_context(tc.tile_pool(name="rcp", bufs=2))
        stg = ctx.enter_context(tc.tile_pool(name="stg", bufs=2))
        psl = ctx.enter_context(tc.tile_pool(name="psl", bufs=2, space="PSUM"))
        psc = ctx.enter_context(tc.tile_pool(name="psc", bufs=3, space="PSUM"))
        psx = ctx.enter_context(tc.tile_pool(name="psx", bufs=1, space="PSUM"))

        b2_sb = consts.tile([33, DG], bf16)
        nc.sync.dma_start(b2_sb, b2[:])
        ones = consts.tile([33, 512], bf16)
        nc.sync.dma_start(ones, one[:])
        em_sb = consts.tile([128, SKC], f32)
        nc.sync.dma_start(em_sb, emaskf[:])
        id_sb = consts.tile([64, 128], bf16)
        nc.sync.dma_start(id_sb, ident[:])

        qt_sb = big.tile([128, CC, S], bf16)        # Q^T: head h at [(h%2)*64, h//2]
        kt_sb = big.tile([128, CC, S], bf16)        # K^T: same layout
        va_sb = big.tile([128, SKC, HG, DH + 1], bf16)  # [v*em, em] per key/head
        cx_sb = big.tile([128, CC, S], bf16)        # normalized ctx^T

        # ones column of va = exp(-1e9*mask) per key
        nc.sync.dma_start(va_sb[:, :, :, DH], emask8[:])

        # ================= phase A: projections =================
        def project_T(w_dram, brow, x_dram, dst_sb):
            """Q^T / K^T [512, 2048] = w_g^T @ x^T, bias via rank-1 matmul."""
            wt = wpool.tile([128, KC, DG], bf16, tag="w", name="wt")
            nc.sync.dma_start(wt, w_dram[:].rearrange("(ko p) c -> p ko c", p=128))
            for blk in range(4):
                xT = xtp.tile([128, KC, 512], bf16, tag="xT", name="xT")
                nc.sync.dma_start(xT, x_dram[:, blk * 512:(blk + 1) * 512]
                                  .rearrange("(ko p) s -> p ko s", p=128))
                for cc in range(CC):
                    ps = psx.tile([128, 512], f32, tag="psx", name="ps")
                    for kc in range(KC):
                        nc.tensor.matmul(ps, lhsT=wt[:, kc, cc * 128:(cc + 1) * 128],
                                         rhs=xT[:, kc, :],
                                         start=(kc == 0), stop=False)
                    nc.tensor.matmul(ps, lhsT=b2_sb[brow:brow + 1, cc * 128:(cc + 1) * 128],
                                     rhs=ones[brow:brow + 1, 0:512],
                                     start=False, stop=True)
                    with nc.allow_low_precision(reason="proj rounded to bf16"):
                        nc.vector.tensor_copy(
                            dst_sb[:, cc, blk * 512:(blk + 1) * 512], ps)

        project_T(wq, 0, xq, qt_sb)
        project_T(wk, 32, xk, kt_sb)

        # V: [keys, 8h*64] scaled by emask per key, interleaved into va
        wvt = wpool.tile([128, KC, DG], bf16, tag="w", name="wvt")
        nc.sync.dma_start(wvt, wv[:].rearrange("(ko p) c -> p ko c", p=128))
        for sc in range(SKC):
            xvt = xtp.tile([128, KC, 128], bf16, tag="xT", name="xvt")
            nc.sync.dma_start(xvt, xv[:, sc * 128:(sc + 1) * 128]
                              .rearrange("(ko p) s -> p ko s", p=128))
            ps = psx.tile([128, 512], f32, tag="psx", name="ps")
            for kc in range(KC):
                nc.tensor.matmul(ps, lhsT=xvt[:, kc, :], rhs=wvt[:, kc, :],
                                 start=(kc == 0), stop=(kc == KC - 1))
            with nc.allow_low_precision(reason="va in bf16"):
                nc.vector.tensor_scalar_mul(
                    va_sb[:, sc, :, 0:DH],
                    ps.rearrange("p (h d) -> p h d", h=HG),
                    em_sb[:, sc:sc + 1])

        # ================= phase B: pipelined attention =================
        state = {}

        def emit_logits_pair(t, kcp):
            st_ = state[t]
            h, sqb = st_["h"], st_["sqb"]
            hp, hcc = (h % 2) * 64, h // 2
            ps_ = psl.tile([128, 1024], f32, tag="psl", name="psl")
            for half in range(2):
                skc = kcp * 2 + half
                nc.tensor.matmul(ps_[:, half * 512:(half + 1) * 512],
                                 lhsT=kt_sb[hp:hp + 64, hcc,
                                            skc * 128:(skc + 1) * 128],
                                 rhs=qt_sb[hp:hp + 64, hcc,
                                           sqb * 512:(sqb + 1) * 512],
                                 start=True, stop=True)
            pt = ptp.tile([128, 2, 512], bf16, tag="pt", name="pt")
            nc.scalar.activation(pt.rearrange("p a b -> p (a b)"), ps_, Exp,
                                 scale=0.125)
            st_["pt"].append(pt)

        def emit_ctx_chunk(t, skc):
            st_ = state[t]
            if skc == 0:
                st_["psc"] = psc.tile([128, 512], f32, tag="psc", name="psc")
            nc.tensor.matmul(st_["psc"][0:DH + 1, :],
                             lhsT=va_sb[:, skc, st_["h"], :],
                             rhs=st_["pt"][skc // 2][:, skc % 2, :],
                             start=(skc == 0), stop=(skc == SKC - 1))

        def emit_norm_dve(t):
            """Issued at iteration start: runs on DVE while the PE streams."""
            st_ = state[t]
            cu = stg.tile([DH + 1, 512], f32, tag="cu", name="cu")
            nc.vector.tensor_copy(cu, st_["psc"][0:DH + 1, :])
            rec = rcp.tile([1, 512], bf16, tag="rec", name="rec")
            with nc.allow_low_precision(reason="recip rounded to bf16"):
                nc.vector.reciprocal(rec, cu[DH:DH + 1, :])
            st_["cu"], st_["rec"] = cu, rec

        def emit_norm_bcast(t):
            """Issued mid-iteration: rec is ready by then, PE never waits."""
            st_ = state[t]
            h, sqb = st_["h"], st_["sqb"]
            hcc, odd = h // 2, h % 2
            cu = st_["cu"]
            bc = psx.tile([128, 512], f32, tag="psx", name="bc")
            nc.tensor.matmul(bc[0:64, :], lhsT=ones[0:1, 0:64],
                             rhs=st_["rec"][:], start=True, stop=True)
            with nc.allow_low_precision(reason="ctxn in bf16"):
                if not odd:
                    dst = cx_sb[0:64, hcc, sqb * 512:(sqb + 1) * 512]
                    nc.vector.tensor_mul(out=dst, in0=cu[0:DH, :],
                                         in1=bc[0:64, :])
                else:
                    tm = tmp.tile([64, 512], bf16, tag="tmp", name="tm")
                    nc.vector.tensor_mul(out=tm, in0=cu[0:DH, :],
                                         in1=bc[0:64, :])
                    st_["tm"] = tm

        def emit_norm_shift(t):
            """Issued at iteration end (odd heads only): tm is long ready."""
            st_ = state[t]
            h, sqb = st_["h"], st_["sqb"]
            hcc = h // 2
            if h % 2:
                sh = psx.tile([128, 512], f32, tag="psx", name="sh")
                nc.tensor.matmul(sh, lhsT=id_sb[:], rhs=st_["tm"][:],
                                 start=True, stop=True)
                dst = cx_sb[64:128, hcc, sqb * 512:(sqb + 1) * 512]
                with nc.allow_low_precision(reason="ctxn in bf16"):
                    nc.vector.tensor_copy(dst, sh[64:128, :])
            del state[t]

        for t in range(NT):
            h, sqb = divmod(t, 4)
            state[t] = {"h": h, "sqb": sqb, "pt": []}
            if t >= 2:
                emit_norm_dve(t - 2)
            for kcp in range(SKC // 2):
                emit_logits_pair(t, kcp)
                if t >= 1:
                    emit_ctx_chunk(t - 1, kcp * 2)
                    emit_ctx_chunk(t - 1, kcp * 2 + 1)
                if t >= 2 and kcp == 5:
                    emit_norm_bcast(t - 2)
            if t >= 2:
                emit_norm_shift(t - 2)
        for skc in range(SKC):
            emit_ctx_chunk(NT - 1, skc)
        for tl in (NT - 2, NT - 1):
            emit_norm_dve(tl)
            emit_norm_bcast(tl)
            emit_norm_shift(tl)

        # ================= phase C: output projection =================
        wot = wpool.tile([128, CC, D], bf16, tag="w", name="wot")
        nc.sync.dma_start(wot, wo[:].rearrange("(co p) c -> p co c", p=128))
        for st8 in range(SKC):
            ot = stg.tile([128, 1024], f32, tag="ost", name="ot")
            for half in range(2):
                ps = psx.tile([128, 512], f32, tag="psx", name="ps")
                for cc in range(CC):
                    nc.tensor.matmul(ps,
                                     lhsT=cx_sb[:, cc, st8 * 128:(st8 + 1) * 128],
                                     rhs=wot[:, cc, half * 512:(half + 1) * 512],
                                     start=(cc == 0), stop=(cc == CC - 1))
                nc.vector.tensor_copy(ot[:, half * 512:(half + 1) * 512], ps)
            nc.sync.dma_start(out[st8 * 128:(st8 + 1) * 128, :], ot)


_NC_CACHE = None


def kernel(query, key, value, mask, wq, bq, wk, bk, wv, bv, wo, bo):
    global _NC_CACHE
    if _NC_CACHE is None:
        _NC_CACHE = _build()
    nc = _NC_CACHE

    query = np.asarray(query, dtype=np.float32)
    key = np.asarray(key, dtype=np.float32)
    value = np.asarray(value, dtype=np.float32)
    mask = np.asarray(mask, dtype=np.float32)
    wq_np = np.asarray(wq, np.float32)
    wk_np = np.asarray(wk, np.float32)
    wv_np = np.asarray(wv, np.float32)
    wo_np = np.asarray(wo, np.float32)
    bq_np = np.asarray(bq, np.float32)
    bk_np = np.asarray(bk, np.float32)
    # fold bv and bo through the output projection (added on host at the end)
    bias_out = (np.asarray(bo, np.float64) +
                np.asarray(bv, np.float64) @ np.asarray(wo_np, np.float64)
                ).astype(np.float32)

    xT = {}
    for b in range(B):
        xT[b] = (np.ascontiguousarray(query[b].T).astype(np_bf16),
                 np.ascontiguousarray(key[b].T).astype(np_bf16),
                 np.ascontiguousarray(value[b].T).astype(np_bf16))
    shared_g = []
    for g in range(2):
        cols = slice(DG * g, DG * (g + 1))
        b2_host = np.zeros((33, DG), np.float32)
        b2_host[0] = bq_np[cols]
        b2_host[32] = bk_np[cols]
        shared_g.append({
            "wq": np.ascontiguousarray(wq_np[:, cols]).astype(np_bf16),
            "wk": np.ascontiguousarray(wk_np[:, cols]).astype(np_bf16),
            "wv": np.ascontiguousarray(wv_np[:, cols]).astype(np_bf16),
            "wo": np.ascontiguousarray(wo_np[cols, :]).astype(np_bf16),
            "b2": b2_host.astype(np_bf16),
        })
    one_host = np.ones((33, 512), np_bf16)
    id_host = np.concatenate([np.zeros((64, 64), np.float32),
                              np.eye(64, dtype=np.float32)],
                             axis=1).astype(np_bf16)

    in_maps = []
    for core in range(N_CORES):
        b, g = divmod(core, 2)
        em = np.exp(mask[b, 0, 0] * np.float32(-1e9)).astype(np.float32)
        emc = np.ascontiguousarray(em.reshape(SKC, 128).T)   # [128, SKC]
        em8 = np.ascontiguousarray(
            np.repeat(emc[:, :, None], HG, axis=2)).astype(np_bf16)
        in_maps.append({
            "xq": xT[b][0], "xk": xT[b][1], "xv": xT[b][2],
            "emask8": em8, "emaskf": emc,
            "one": one_host, "ident": id_host,
            **shared_g[g],
        })

    res = run_bass_kernel_spmd(nc, in_maps, core_ids=list(range(N_CORES)))
    full = np.empty((B, S, D), np.float32)
    for b in range(B):
        full[b] = res.results[2 * b]["out"]
        full[b] += res.results[2 * b + 1]["out"]
        full[b] += bias_out
    return full


# revision 15
# speedup vs baseline: 1.5410x; 1.5410x over previous
"""MultiHeadAttention Trainium2 kernel.

Full inputs -> full output. Sharding: 8 cores = (batch b in 0..3) x (head
group g in 0..1, 8 heads each). Each core projects Q/K/V for its head group
over all 2048 positions of batch b, runs attention for its 8 heads, applies
its half of the output projection (wo rows for its heads), and returns a
partial [2048, 1024] output. Host: full[b] = part(b,0) + part(b,1) + bias.

Uniform bf16 datapath (fp32 PSUM accumulation), everything SBUF-resident
between phases (no DRAM round trips) to stay under the chip power envelope:
  phase A: project Q^T, K^T [512, 2048] (rank-1 bias matmuls) and the
           V table va [keys, head, 65] with a ones column (so the softmax
           denominator falls out of the ctx matmul) scaled by
           exp(-1e9*mask) per key (exact mask semantics at zero cost).
  phase B: 32 software-pipelined (head, query-block) iterations:
           logits^T [sk, 512q] = K_h^T.T @ Q_h^T, P = exp(0.125*l) on the
           scalar engine -> bf16, ctx matmuls accumulate [65, 512] (row 64
           = denominator), DVE fast-reciprocal, PE rank-1 broadcast of the
           recips, DVE normalize-mul -> ctxn bf16 (odd heads go through a
           shifted-identity matmul to land at partitions 64..127).
  phase C: out_partial = ctxn @ wo, f32 partials to DRAM.
"""

import numpy as np
import ml_dtypes

import concourse.bass as bass
import concourse.mybir as mybir
import concourse.tile as tile
from concourse import bacc
from concourse.bass_utils import run_bass_kernel_spmd

f32 = mybir.dt.float32
bf16 = mybir.dt.bfloat16
np_bf16 = ml_dtypes.bfloat16

B, S, D, H, DH = 4, 2048, 1024, 16, 64
HG = H // 2          # 8 heads per core
DG = HG * DH         # 512 projection cols per core
N_CORES = 8
Exp = mybir.ActivationFunctionType.Exp

KC = D // 128        # 8 contraction chunks over d_model
CC = DG // 128       # 4 chunks over the head-group dim
SKC = S // 128       # 16 key chunks
NT = HG * 4          # 32 pipelined iterations (head, 512-query block)


def _build():
    nc = bacc.Bacc(None, target_bir_lowering=False)

    xq = nc.dram_tensor("xq", [D, S], bf16, kind="ExternalInput")   # query^T
    xk = nc.dram_tensor("xk", [D, S], bf16, kind="ExternalInput")   # key^T
    xv = nc.dram_tensor("xv", [D, S], bf16, kind="ExternalInput")   # value^T
    wq = nc.dram_tensor("wq", [D, DG], bf16, kind="ExternalInput")
    wk = nc.dram_tensor("wk", [D, DG], bf16, kind="ExternalInput")
    wv = nc.dram_tensor("wv", [D, DG], bf16, kind="ExternalInput")
    wo = nc.dram_tensor("wo", [DG, D], bf16, kind="ExternalInput")
    b2 = nc.dram_tensor("b2", [33, DG], bf16, kind="ExternalInput")  # bq@0, bk@32
    one = nc.dram_tensor("one", [33, 512], bf16, kind="ExternalInput")
    emask8 = nc.dram_tensor("emask8", [128, SKC, HG], bf16, kind="ExternalInput")
    emaskf = nc.dram_tensor("emaskf", [128, SKC], f32, kind="ExternalInput")
    ident = nc.dram_tensor("ident", [64, 128], bf16, kind="ExternalInput")
    out = nc.dram_tensor("out", [S, D], f32, kind="ExternalOutput")

    with tile.TileContext(nc) as tc:
        _emit(nc, tc, xq, xk, xv, wq, wk, wv, wo, b2, one, emask8, emaskf,
              ident, out)
    nc.finalize()
    return nc


def _emit(nc, tc, xq, xk, xv, wq, wk, wv, wo, b2, one, emask8, emaskf,
          ident, out):
    from contextlib import ExitStack

    with ExitStack() as ctx:
        consts = ctx.enter_context(tc.tile_pool(name="consts", bufs=1))
        wpool = ctx.enter_context(tc.tile_pool(name="wpool", bufs=2))
        xtp = ctx.enter_context(tc.tile_pool(name="xtp", bufs=2))
        big = ctx.enter_context(tc.tile_pool(name="big", bufs=1))
        ptp = ctx.enter_context(tc.tile_pool(name="ptp", bufs=16))
        tmp = ctx.enter_context(tc.tile_pool(name="tmp", bufs=1))
        rcp = ctx.enter_context(tc.tile_pool(name="rcp", bufs=3))
        stg = ctx.enter_context(tc.tile_pool(name="stg", bufs=2))
        psl = ctx.enter_context(tc.tile_pool(name="psl", bufs=2, space="PSUM"))
        psc = ctx.enter_context(tc.tile_pool(name="psc", bufs=3, space="PSUM"))
        psx = ctx.enter_context(tc.tile_pool(name="psx", bufs=1, space="PSUM"))

        b2_sb = consts.tile([33, DG], bf16)
        nc.sync.dma_start(b2_sb, b2[:])
        ones = consts.tile([33, 512], bf16)
        nc.sync.dma_start(ones, one[:])
        em_sb = consts.tile([128, SKC], f32)
        nc.sync.dma_start(em_sb, emaskf[:])
        id_sb = consts.tile([64, 128], bf16)
        nc.sync.dma_start(id_sb, ident[:])

        qt_sb = big.tile([128, CC, S], bf16)        # Q^T: head h at [(h%2)*64, h//2]
        kt_sb = big.tile([128, CC, S], bf16)        # K^T: same layout
        va_sb = big.tile([128, SKC, HG, DH + 1], bf16)  # [v*em, em] per key/head
        cx_sb = big.tile([128, CC, S], bf16)        # normalized ctx^T

        # ones column of va = exp(-1e9*mask) per key
        nc.sync.dma_start(va_sb[:, :, :, DH], emask8[:])

        # ================= phase A: projections =================
        def project_T(w_dram, brow, x_dram, dst_sb):
            """Q^T / K^T [512, 2048] = w_g^T @ x^T, bias via rank-1 matmul."""
            wt = wpool.tile([128, KC, DG], bf16, tag="w", name="wt")
            nc.sync.dma_start(wt, w_dram[:].rearrange("(ko p) c -> p ko c", p=128))
            for blk in range(4):
                xT = xtp.tile([128, KC, 512], bf16, tag="xT", name="xT")
                nc.sync.dma_start(xT, x_dram[:, blk * 512:(blk + 1) * 512]
                                  .rearrange("(ko p) s -> p ko s", p=128))
                for cc in range(CC):
                    ps = psl.tile([128, 512], f32, tag="psl", name="ps")
                    for kc in range(KC):
                        nc.tensor.matmul(ps, lhsT=wt[:, kc, cc * 128:(cc + 1) * 128],
                                         rhs=xT[:, kc, :],
                                         start=(kc == 0), stop=False)
                    nc.tensor.matmul(ps, lhsT=b2_sb[brow:brow + 1, cc * 128:(cc + 1) * 128],
                                     rhs=ones[brow:brow + 1, 0:512],
                                     start=False, stop=True)
                    with nc.allow_low_precision(reason="proj rounded to bf16"):
                        nc.vector.tensor_copy(
                            dst_sb[:, cc, blk * 512:(blk + 1) * 512], ps)

        project_T(wq, 0, xq, qt_sb)
        project_T(wk, 32, xk, kt_sb)

        # V: [keys, 8h*64] scaled by emask per key, interleaved into va
        wvt = wpool.tile([128, KC, DG], bf16, tag="w", name="wvt")
        nc.sync.dma_start(wvt, wv[:].rearrange("(ko p) c -> p ko c", p=128))
        for sc in range(SKC):
            xvt = xtp.tile([128, KC, 128], bf16, tag="xT", name="xvt")
            nc.sync.dma_start(xvt, xv[:, sc * 128:(sc + 1) * 128]
                              .rearrange("(ko p) s -> p ko s", p=128))
            ps = psl.tile([128, 512], f32, tag="psl", name="ps")
            for kc in range(KC):
                nc.tensor.matmul(ps, lhsT=xvt[:, kc, :], rhs=wvt[:, kc, :],
                                 start=(kc == 0), stop=(kc == KC - 1))
            with nc.allow_low_precision(reason="va in bf16"):
                nc.vector.tensor_scalar_mul(
                    va_sb[:, sc, :, 0:DH],
                    ps.rearrange("p (h d) -> p h d", h=HG),
                    em_sb[:, sc:sc + 1])

        # ================= phase B: pipelined attention =================
        state = {}

        def emit_logits_pair(t, kcp):
            st_ = state[t]
            h, sqb = st_["h"], st_["sqb"]
            hp, hcc = (h % 2) * 64, h // 2
            ps_ = psl.tile([128, 1024], f32, tag="psl", name="psl")
            for half in range(2):
                skc = kcp * 2 + half
                nc.tensor.matmul(ps_[:, half * 512:(half + 1) * 512],
                                 lhsT=kt_sb[hp:hp + 64, hcc,
                                            skc * 128:(skc + 1) * 128],
                                 rhs=qt_sb[hp:hp + 64, hcc,
                                           sqb * 512:(sqb + 1) * 512],
                                 start=True, stop=True)
            pt = ptp.tile([128, 2, 512], bf16, tag="pt", name="pt")
            nc.scalar.activation(pt.rearrange("p a b -> p (a b)"), ps_, Exp,
                                 scale=0.125)
            st_["pt"].append(pt)

        def emit_ctx_chunk(t, skc):
            st_ = state[t]
            if skc == 0:
                st_["psc"] = psc.tile([128, 512], f32, tag="psc", name="psc")
            nc.tensor.matmul(st_["psc"][0:DH + 1, :],
                             lhsT=va_sb[:, skc, st_["h"], :],
                             rhs=st_["pt"][skc // 2][:, skc % 2, :],
                             start=(skc == 0), stop=(skc == SKC - 1))

        def emit_norm_dve(t):
            """Issued at iteration start: runs on DVE while the PE streams."""
            st_ = state[t]
            cu = stg.tile([DH + 1, 512], f32, tag="cu", name="cu")
            nc.vector.tensor_copy(cu, st_["psc"][0:DH + 1, :])
            den = rcp.tile([1, 512], f32, tag="den", name="den")
            nc.vector.tensor_copy(den, st_["psc"][DH:DH + 1, :])
            recf = rcp.tile([1, 512], f32, tag="recf", name="recf")
            nc.vector.reciprocal_approx_fast(recf, den)
            rec = rcp.tile([1, 512], bf16, tag="rec", name="rec")
            with nc.allow_low_precision(reason="recip rounded to bf16"):
                nc.vector.tensor_copy(rec, recf)
            st_["cu"], st_["rec"] = cu, rec

        def emit_norm_bcast(t):
            """Issued mid-iteration: rec is ready by then, PE never waits."""
            st_ = state[t]
            h, sqb = st_["h"], st_["sqb"]
            hcc, odd = h // 2, h % 2
            cu = st_["cu"]
            bc = psx.tile([128, 512], f32, tag="psx", name="bc")
            nc.tensor.matmul(bc[0:64, :], lhsT=ones[0:1, 0:64],
                             rhs=st_["rec"][:], start=True, stop=True)
            with nc.allow_low_precision(reason="ctxn in bf16"):
                if not odd:
                    dst = cx_sb[0:64, hcc, sqb * 512:(sqb + 1) * 512]
                    nc.vector.tensor_mul(out=dst, in0=cu[0:DH, :],
                                         in1=bc[0:64, :])
                else:
                    tm = tmp.tile([64, 512], bf16, tag="tmp", name="tm")
                    nc.vector.tensor_mul(out=tm, in0=cu[0:DH, :],
                                         in1=bc[0:64, :])
                    st_["tm"] = tm

        def emit_norm_shift(t):
            """Issued at iteration end (odd heads only): tm is long ready."""
            st_ = state[t]
            h, sqb = st_["h"], st_["sqb"]
            hcc = h // 2
            if h % 2:
                sh = psx.tile([128, 512], f32, tag="psx", name="sh")
                nc.tensor.matmul(sh, lhsT=id_sb[:], rhs=st_["tm"][:],
                                 start=True, stop=True)
                dst = cx_sb[64:128, hcc, sqb * 512:(sqb + 1) * 512]
                with nc.allow_low_precision(reason="ctxn in bf16"):
                    nc.vector.tensor_copy(dst, sh[64:128, :])
            del state[t]

        for t in range(NT):
            h, sqb = divmod(t, 4)
            state[t] = {"h": h, "sqb": sqb, "pt": []}
            if t >= 2:
                emit_norm_dve(t - 2)
            for kcp in range(SKC // 2):
                emit_logits_pair(t, kcp)
                if t >= 1:
                    emit_ctx_chunk(t - 1, kcp * 2)
                    emit_ctx_chunk(t - 1, kcp * 2 + 1)
                if t >= 2 and kcp == 5:
                    emit_norm_bcast(t - 2)
            if t >= 2:
                emit_norm_shift(t - 2)
        for skc in range(SKC):
            emit_ctx_chunk(NT - 1, skc)
        for tl in (NT - 2, NT - 1):
            emit_norm_dve(tl)
            emit_norm_bcast(tl)
            emit_norm_shift(tl)

        # ================= phase C: output projection =================
        wot = wpool.tile([128, CC, D], bf16, tag="w", name="wot")
        nc.sync.dma_start(wot, wo[:].rearrange("(co p) c -> p co c", p=128))
        for st8 in range(SKC):
            ot = stg.tile([128, 1024], f32, tag="ost", name="ot")
            for half in range(2):
                ps = psl.tile([128, 512], f32, tag="psl", name="ps")
                for cc in range(CC):
                    nc.tensor.matmul(ps,
                                     lhsT=cx_sb[:, cc, st8 * 128:(st8 + 1) * 128],
                                     rhs=wot[:, cc, half * 512:(half + 1) * 512],
                                     start=(cc == 0), stop=(cc == CC - 1))
                nc.vector.tensor_copy(ot[:, half * 512:(half + 1) * 512], ps)
            nc.sync.dma_start(out[st8 * 128:(st8 + 1) * 128, :], ot)


_NC_CACHE = None


def kernel(query, key, value, mask, wq, bq, wk, bk, wv, bv, wo, bo):
    global _NC_CACHE
    if _NC_CACHE is None:
        _NC_CACHE = _build()
    nc = _NC_CACHE

    query = np.asarray(query, dtype=np.float32)
    key = np.asarray(key, dtype=np.float32)
    value = np.asarray(value, dtype=np.float32)
    mask = np.asarray(mask, dtype=np.float32)
    wq_np = np.asarray(wq, np.float32)
    wk_np = np.asarray(wk, np.float32)
    wv_np = np.asarray(wv, np.float32)
    wo_np = np.asarray(wo, np.float32)
    bq_np = np.asarray(bq, np.float32)
    bk_np = np.asarray(bk, np.float32)
    # fold bv and bo through the output projection (added on host at the end)
    bias_out = (np.asarray(bo, np.float64) +
                np.asarray(bv, np.float64) @ np.asarray(wo_np, np.float64)
                ).astype(np.float32)

    xT = {}
    for b in range(B):
        xT[b] = (np.ascontiguousarray(query[b].T).astype(np_bf16),
                 np.ascontiguousarray(key[b].T).astype(np_bf16),
                 np.ascontiguousarray(value[b].T).astype(np_bf16))
    shared_g = []
    for g in range(2):
        cols = slice(DG * g, DG * (g + 1))
        b2_host = np.zeros((33, DG), np.float32)
        b2_host[0] = bq_np[cols]
        b2_host[32] = bk_np[cols]
        shared_g.append({
            "wq": np.ascontiguousarray(wq_np[:, cols]).astype(np_bf16),
            "wk": np.ascontiguousarray(wk_np[:, cols]).astype(np_bf16),
            "wv": np.ascontiguousarray(wv_np[:, cols]).astype(np_bf16),
            "wo": np.ascontiguousarray(wo_np[cols, :]).astype(np_bf16),
            "b2": b2_host.astype(np_bf16),
        })
    one_host = np.ones((33, 512), np_bf16)
    id_host = np.concatenate([np.zeros((64, 64), np.float32),
                              np.eye(64, dtype=np.float32)],
                             axis=1).astype(np_bf16)

    in_maps = []
    for core in range(N_CORES):
        b, g = divmod(core, 2)
        em = np.exp(mask[b, 0, 0] * np.float32(-1e9)).astype(np.float32)
        emc = np.ascontiguousarray(em.reshape(SKC, 128).T)   # [128, SKC]
        em8 = np.ascontiguousarray(
            np.repeat(emc[:, :, None], HG, axis=2)).astype(np_bf16)
        in_maps.append({
            "xq": xT[b][0], "xk": xT[b][1], "xv": xT[b][2],
            "emask8": em8, "emaskf": emc,
            "one": one_host, "ident": id_host,
            **shared_g[g],
        })

    res = run_bass_kernel_spmd(nc, in_maps, core_ids=list(range(N_CORES)))
    full = np.empty((B, S, D), np.float32)
    for b in range(B):
        full[b] = res.results[2 * b]["out"]
        full[b] += res.results[2 * b + 1]["out"]
        full[b] += bias_out
    return full


# revision 16
# speedup vs baseline: 1.5732x; 1.0209x over previous
"""MultiHeadAttention Trainium2 kernel.

Full inputs -> full output. Sharding: 8 cores = (batch b in 0..3) x (head
group g in 0..1, 8 heads each). Each core projects Q/K/V for its head group
over all 2048 positions of batch b, runs attention for its 8 heads, applies
its half of the output projection (wo rows for its heads), and returns a
partial [2048, 1024] output. Host: full[b] = part(b,0) + part(b,1) + bias.

Uniform bf16 datapath (fp32 PSUM accumulation), everything SBUF-resident
between phases (no DRAM round trips) to stay under the chip power envelope:
  phase A: project Q^T, K^T [512, 2048] (rank-1 bias matmuls) and the
           V table va [keys, head, 65] with a ones column (so the softmax
           denominator falls out of the ctx matmul) scaled by
           exp(-1e9*mask) per key (exact mask semantics at zero cost).
  phase B: 32 software-pipelined (head, query-block) iterations:
           logits^T [sk, 512q] = K_h^T.T @ Q_h^T, P = exp(0.125*l) on the
           scalar engine -> bf16, ctx matmuls accumulate [65, 512] (row 64
           = denominator), DVE fast-reciprocal, PE rank-1 broadcast of the
           recips, DVE normalize-mul -> ctxn bf16 (odd heads go through a
           shifted-identity matmul to land at partitions 64..127).
  phase C: out_partial = ctxn @ wo, f32 partials to DRAM.
"""

import numpy as np
import ml_dtypes

import concourse.bass as bass
import concourse.mybir as mybir
import concourse.tile as tile
from concourse import bacc
from concourse.bass_utils import run_bass_kernel_spmd

f32 = mybir.dt.float32
bf16 = mybir.dt.bfloat16
np_bf16 = ml_dtypes.bfloat16

B, S, D, H, DH = 4, 2048, 1024, 16, 64
HG = H // 2          # 8 heads per core
DG = HG * DH         # 512 projection cols per core
N_CORES = 8
Exp = mybir.ActivationFunctionType.Exp

KC = D // 128        # 8 contraction chunks over d_model
CC = DG // 128       # 4 chunks over the head-group dim
SKC = S // 128       # 16 key chunks
NT = HG * 4          # 32 pipelined iterations (head, 512-query block)


def _build():
    nc = bacc.Bacc(None, target_bir_lowering=False)

    xq = nc.dram_tensor("xq", [D, S], bf16, kind="ExternalInput")   # query^T
    xk = nc.dram_tensor("xk", [D, S], bf16, kind="ExternalInput")   # key^T
    xv = nc.dram_tensor("xv", [D, S], bf16, kind="ExternalInput")   # value^T
    wq = nc.dram_tensor("wq", [D, DG], bf16, kind="ExternalInput")
    wk = nc.dram_tensor("wk", [D, DG], bf16, kind="ExternalInput")
    wv = nc.dram_tensor("wv", [D, DG], bf16, kind="ExternalInput")
    wo = nc.dram_tensor("wo", [DG, D], bf16, kind="ExternalInput")
    b2 = nc.dram_tensor("b2", [33, DG], bf16, kind="ExternalInput")  # bq@0, bk@32
    one = nc.dram_tensor("one", [33, 512], bf16, kind="ExternalInput")
    emask8 = nc.dram_tensor("emask8", [128, SKC, HG], bf16, kind="ExternalInput")
    emaskf = nc.dram_tensor("emaskf", [128, SKC], f32, kind="ExternalInput")
    ident = nc.dram_tensor("ident", [64, 128], bf16, kind="ExternalInput")
    out = nc.dram_tensor("out", [S, D], f32, kind="ExternalOutput")

    with tile.TileContext(nc) as tc:
        _emit(nc, tc, xq, xk, xv, wq, wk, wv, wo, b2, one, emask8, emaskf,
              ident, out)
    nc.finalize()
    return nc


def _emit(nc, tc, xq, xk, xv, wq, wk, wv, wo, b2, one, emask8, emaskf,
          ident, out):
    from contextlib import ExitStack

    with ExitStack() as ctx:
        consts = ctx.enter_context(tc.tile_pool(name="consts", bufs=1))
        wpool = ctx.enter_context(tc.tile_pool(name="wpool", bufs=2))
        xtp = ctx.enter_context(tc.tile_pool(name="xtp", bufs=2))
        big = ctx.enter_context(tc.tile_pool(name="big", bufs=1))
        ptp = ctx.enter_context(tc.tile_pool(name="ptp", bufs=16))
        tmp = ctx.enter_context(tc.tile_pool(name="tmp", bufs=1))
        rcp = ctx.enter_context(tc.tile_pool(name="rcp", bufs=3))
        stg = ctx.enter_context(tc.tile_pool(name="stg", bufs=2))
        psl = ctx.enter_context(tc.tile_pool(name="psl", bufs=2, space="PSUM"))
        psc = ctx.enter_context(tc.tile_pool(name="psc", bufs=3, space="PSUM"))
        psx = ctx.enter_context(tc.tile_pool(name="psx", bufs=1, space="PSUM"))

        wq0 = wpool.tile([128, KC, DG], bf16, tag="w", name="wq0")
        nc.sync.dma_start(wq0, wq[:].rearrange("(ko p) c -> p ko c", p=128))
        xq0 = xtp.tile([128, KC, 512], bf16, tag="xT", name="xq0")
        nc.sync.dma_start(xq0, xq[:, 0:512]
                          .rearrange("(ko p) s -> p ko s", p=128))

        b2_sb = consts.tile([33, DG], bf16)
        nc.sync.dma_start(b2_sb, b2[:])
        ones = consts.tile([33, 512], bf16)
        nc.sync.dma_start(ones, one[:])
        em_sb = consts.tile([128, SKC], f32)
        nc.sync.dma_start(em_sb, emaskf[:])
        id_sb = consts.tile([64, 128], bf16)
        nc.sync.dma_start(id_sb, ident[:])

        qt_sb = big.tile([128, CC, S], bf16)        # Q^T: head h at [(h%2)*64, h//2]
        kt_sb = big.tile([128, CC, S], bf16)        # K^T: same layout
        va_sb = big.tile([128, SKC, HG, DH + 1], bf16)  # [v*em, em] per key/head
        cx_sb = big.tile([128, CC, S], bf16)        # normalized ctx^T

        # ones column of va = exp(-1e9*mask) per key
        nc.sync.dma_start(va_sb[:, :, :, DH], emask8[:])

        # ================= phase A: projections =================
        def project_T(w_dram, brow, x_dram, dst_sb, wt=None, x0=None):
            """Q^T / K^T [512, 2048] = w_g^T @ x^T, bias via rank-1 matmul."""
            if wt is None:
                wt = wpool.tile([128, KC, DG], bf16, tag="w", name="wt")
                nc.sync.dma_start(wt, w_dram[:].rearrange("(ko p) c -> p ko c", p=128))
            for blk in range(4):
                if blk == 0 and x0 is not None:
                    xT = x0
                else:
                    xT = xtp.tile([128, KC, 512], bf16, tag="xT", name="xT")
                    nc.sync.dma_start(xT, x_dram[:, blk * 512:(blk + 1) * 512]
                                      .rearrange("(ko p) s -> p ko s", p=128))
                for cc in range(CC):
                    ps = psl.tile([128, 512], f32, tag="psl", name="ps")
                    for kc in range(KC):
                        nc.tensor.matmul(ps, lhsT=wt[:, kc, cc * 128:(cc + 1) * 128],
                                         rhs=xT[:, kc, :],
                                         start=(kc == 0), stop=False)
                    nc.tensor.matmul(ps, lhsT=b2_sb[brow:brow + 1, cc * 128:(cc + 1) * 128],
                                     rhs=ones[brow:brow + 1, 0:512],
                                     start=False, stop=True)
                    with nc.allow_low_precision(reason="proj rounded to bf16"):
                        nc.vector.tensor_copy(
                            dst_sb[:, cc, blk * 512:(blk + 1) * 512], ps)

        project_T(wq, 0, xq, qt_sb, wt=wq0, x0=xq0)
        project_T(wk, 32, xk, kt_sb)

        # V: [keys, 8h*64] scaled by emask per key, interleaved into va
        wvt = wpool.tile([128, KC, DG], bf16, tag="w", name="wvt")
        nc.sync.dma_start(wvt, wv[:].rearrange("(ko p) c -> p ko c", p=128))
        for sc in range(SKC):
            xvt = xtp.tile([128, KC, 128], bf16, tag="xT", name="xvt")
            nc.sync.dma_start(xvt, xv[:, sc * 128:(sc + 1) * 128]
                              .rearrange("(ko p) s -> p ko s", p=128))
            ps = psl.tile([128, 512], f32, tag="psl", name="ps")
            for kc in range(KC):
                nc.tensor.matmul(ps, lhsT=xvt[:, kc, :], rhs=wvt[:, kc, :],
                                 start=(kc == 0), stop=(kc == KC - 1))
            with nc.allow_low_precision(reason="va in bf16"):
                nc.vector.tensor_scalar_mul(
                    va_sb[:, sc, :, 0:DH],
                    ps.rearrange("p (h d) -> p h d", h=HG),
                    em_sb[:, sc:sc + 1])

        # ================= phase B: pipelined attention =================
        state = {}

        def emit_logits_pair(t, kcp):
            st_ = state[t]
            h, sqb = st_["h"], st_["sqb"]
            hp, hcc = (h % 2) * 64, h // 2
            ps_ = psl.tile([128, 1024], f32, tag="psl", name="psl")
            for half in range(2):
                skc = kcp * 2 + half
                nc.tensor.matmul(ps_[:, half * 512:(half + 1) * 512],
                                 lhsT=kt_sb[hp:hp + 64, hcc,
                                            skc * 128:(skc + 1) * 128],
                                 rhs=qt_sb[hp:hp + 64, hcc,
                                           sqb * 512:(sqb + 1) * 512],
                                 start=True, stop=True)
            pt = ptp.tile([128, 2, 512], bf16, tag="pt", name="pt")
            nc.scalar.activation(pt.rearrange("p a b -> p (a b)"), ps_, Exp,
                                 scale=0.125)
            st_["pt"].append(pt)

        def emit_ctx_chunk(t, skc):
            st_ = state[t]
            if skc == 0:
                st_["psc"] = psc.tile([128, 512], f32, tag="psc", name="psc")
            nc.tensor.matmul(st_["psc"][0:DH + 1, :],
                             lhsT=va_sb[:, skc, st_["h"], :],
                             rhs=st_["pt"][skc // 2][:, skc % 2, :],
                             start=(skc == 0), stop=(skc == SKC - 1))

        def emit_norm_dve(t):
            """Issued at iteration start: runs on DVE while the PE streams."""
            st_ = state[t]
            cu = stg.tile([DH + 1, 512], f32, tag="cu", name="cu")
            nc.vector.tensor_copy(cu, st_["psc"][0:DH + 1, :])
            den = rcp.tile([1, 512], f32, tag="den", name="den")
            nc.vector.tensor_copy(den, st_["psc"][DH:DH + 1, :])
            recf = rcp.tile([1, 512], f32, tag="recf", name="recf")
            nc.vector.reciprocal_approx_fast(recf, den)
            rec = rcp.tile([1, 512], bf16, tag="rec", name="rec")
            with nc.allow_low_precision(reason="recip rounded to bf16"):
                nc.vector.tensor_copy(rec, recf)
            st_["cu"], st_["rec"] = cu, rec

        def emit_norm_bcast(t):
            """Issued mid-iteration: rec is ready by then, PE never waits."""
            st_ = state[t]
            h, sqb = st_["h"], st_["sqb"]
            hcc, odd = h // 2, h % 2
            cu = st_["cu"]
            bc = psx.tile([128, 512], f32, tag="psx", name="bc")
            nc.tensor.matmul(bc[0:64, :], lhsT=ones[0:1, 0:64],
                             rhs=st_["rec"][:], start=True, stop=True)
            with nc.allow_low_precision(reason="ctxn in bf16"):
                if not odd:
                    dst = cx_sb[0:64, hcc, sqb * 512:(sqb + 1) * 512]
                    nc.vector.tensor_mul(out=dst, in0=cu[0:DH, :],
                                         in1=bc[0:64, :])
                else:
                    tm = tmp.tile([64, 512], bf16, tag="tmp", name="tm")
                    nc.vector.tensor_mul(out=tm, in0=cu[0:DH, :],
                                         in1=bc[0:64, :])
                    st_["tm"] = tm

        def emit_norm_shift(t):
            """Issued at iteration end (odd heads only): tm is long ready."""
            st_ = state[t]
            h, sqb = st_["h"], st_["sqb"]
            hcc = h // 2
            if h % 2:
                sh = psx.tile([128, 512], f32, tag="psx", name="sh")
                nc.tensor.matmul(sh, lhsT=id_sb[:], rhs=st_["tm"][:],
                                 start=True, stop=True)
                dst = cx_sb[64:128, hcc, sqb * 512:(sqb + 1) * 512]
                with nc.allow_low_precision(reason="ctxn in bf16"):
                    nc.vector.tensor_copy(dst, sh[64:128, :])
            del state[t]

        for t in range(NT):
            h, sqb = divmod(t, 4)
            state[t] = {"h": h, "sqb": sqb, "pt": []}
            if t >= 2:
                emit_norm_dve(t - 2)
            for kcp in range(SKC // 2):
                emit_logits_pair(t, kcp)
                if t >= 1:
                    emit_ctx_chunk(t - 1, kcp * 2)
                    emit_ctx_chunk(t - 1, kcp * 2 + 1)
                if t >= 2 and kcp == 5:
                    emit_norm_bcast(t - 2)
            if t >= 2:
                emit_norm_shift(t - 2)
        for skc in range(SKC):
            emit_ctx_chunk(NT - 1, skc)
        for tl in (NT - 2, NT - 1):
            emit_norm_dve(tl)
            emit_norm_bcast(tl)
            emit_norm_shift(tl)

        # ================= phase C: output projection =================
        wot = wpool.tile([128, CC, D], bf16, tag="w", name="wot")
        nc.sync.dma_start(wot, wo[:].rearrange("(co p) c -> p co c", p=128))
        for st8 in range(SKC):
            ot = stg.tile([128, 1024], f32, tag="ost", name="ot")
            for half in range(2):
                ps = psl.tile([128, 512], f32, tag="psl", name="ps")
                for cc in range(CC):
                    nc.tensor.matmul(ps,
                                     lhsT=cx_sb[:, cc, st8 * 128:(st8 + 1) * 128],
                                     rhs=wot[:, cc, half * 512:(half + 1) * 512],
                                     start=(cc == 0), stop=(cc == CC - 1))
                nc.vector.tensor_copy(ot[:, half * 512:(half + 1) * 512], ps)
            nc.sync.dma_start(out[st8 * 128:(st8 + 1) * 128, :], ot)


_NC_CACHE = None


def kernel(query, key, value, mask, wq, bq, wk, bk, wv, bv, wo, bo):
    global _NC_CACHE
    if _NC_CACHE is None:
        _NC_CACHE = _build()
    nc = _NC_CACHE

    query = np.asarray(query, dtype=np.float32)
    key = np.asarray(key, dtype=np.float32)
    value = np.asarray(value, dtype=np.float32)
    mask = np.asarray(mask, dtype=np.float32)
    wq_np = np.asarray(wq, np.float32)
    wk_np = np.asarray(wk, np.float32)
    wv_np = np.asarray(wv, np.float32)
    wo_np = np.asarray(wo, np.float32)
    bq_np = np.asarray(bq, np.float32)
    bk_np = np.asarray(bk, np.float32)
    # fold bv and bo through the output projection (added on host at the end)
    bias_out = (np.asarray(bo, np.float64) +
                np.asarray(bv, np.float64) @ np.asarray(wo_np, np.float64)
                ).astype(np.float32)

    xT = {}
    for b in range(B):
        xT[b] = (np.ascontiguousarray(query[b].T).astype(np_bf16),
                 np.ascontiguousarray(key[b].T).astype(np_bf16),
                 np.ascontiguousarray(value[b].T).astype(np_bf16))
    shared_g = []
    for g in range(2):
        cols = slice(DG * g, DG * (g + 1))
        b2_host = np.zeros((33, DG), np.float32)
        b2_host[0] = bq_np[cols]
        b2_host[32] = bk_np[cols]
        shared_g.append({
            "wq": np.ascontiguousarray(wq_np[:, cols]).astype(np_bf16),
            "wk": np.ascontiguousarray(wk_np[:, cols]).astype(np_bf16),
            "wv": np.ascontiguousarray(wv_np[:, cols]).astype(np_bf16),
            "wo": np.ascontiguousarray(wo_np[cols, :]).astype(np_bf16),
            "b2": b2_host.astype(np_bf16),
        })
    one_host = np.ones((33, 512), np_bf16)
    id_host = np.concatenate([np.zeros((64, 64), np.float32),
                              np.eye(64, dtype=np.float32)],
                             axis=1).astype(np_bf16)

    in_maps = []
    for core in range(N_CORES):
        b, g = divmod(core, 2)
        em = np.exp(mask[b, 0, 0] * np.float32(-1e9)).astype(np.float32)
        emc = np.ascontiguousarray(em.reshape(SKC, 128).T)   # [128, SKC]
        em8 = np.ascontiguousarray(
            np.repeat(emc[:, :, None], HG, axis=2)).astype(np_bf16)
        in_maps.append({
            "xq": xT[b][0], "xk": xT[b][1], "xv": xT[b][2],
            "emask8": em8, "emaskf": emc,
            "one": one_host, "ident": id_host,
            **shared_g[g],
        })

    res = run_bass_kernel_spmd(nc, in_maps, core_ids=list(range(N_CORES)))
    full = np.empty((B, S, D), np.float32)
    for b in range(B):
        full[b] = res.results[2 * b]["out"]
        full[b] += res.results[2 * b + 1]["out"]
        full[b] += bias_out
    return full
